# revision 16
# baseline (speedup 1.0000x reference)
"""Causal self-attention (B=2, T=2048, C=2048, NH=16) on 8 TRN2 NeuronCores.

Megatron-style tensor parallelism over heads: each core owns 2 heads.
Per core:
  phase 1: QKV projection in feature-major layout (fp32r matmuls),
           q^T/k^T ([d, tokens]) and V ([tokens, d]) spilled to DRAM.
  phase 2: causal attention per (batch, head) pair computed transposed:
           S^T[k,q] tiles = k^T_tile.T @ q^T_chunk, exp on ScalarE
           (PSUM->SBUF), 0/1 causal mask multiply on diagonal tiles,
           softmax denominator via an all-ones matmul (partition-dim sum),
           O^T[d,q] = V.T-free accumulation over k-tiles, divide by denom.
  phase 3: per-batch AllGather of y^T across cores (4MB shards), then each
           core computes its 256 output channels: out[t, o_slice].
Host side: transpose/shard inputs, concat per-core output column slices.
"""

import numpy as np

import concourse.bacc as bacc
import concourse.mybir as mybir
import concourse.tile as tile
from concourse.bass_utils import run_bass_kernel_spmd
from concourse.hw_specs import get_activation_tables as _get_act_tables


def _act_tables_pin_exp_ln(arch):
    """Resolve Exp and Ln only via the combined natural_log_exp set.

    The default greedy set choice puts Exp in exp_and_others and Ln in
    natural_log, so a kernel alternating exp/ln pays a ~1.3us ACT table
    load per switch. Keys/order are preserved (set ids are positional).
    """
    t = _get_act_tables(arch)
    for name, fns in t.items():
        if name != "natural_log_exp_and_others":
            fns.discard(mybir.ActivationFunctionType.Exp)
            fns.discard(mybir.ActivationFunctionType.Ln)
    return t


bacc.get_activation_tables = _act_tables_pin_exp_ln

F32R = mybir.dt.float32r
F32 = mybir.dt.float32
EXP = mybir.ActivationFunctionType.Exp
LN = mybir.ActivationFunctionType.Ln

B, T, C, NH, HS = 2, 2048, 2048, 16, 128
NCORES = 8
HPC = NH // NCORES          # heads per core
BT = B * T                  # 4096 tokens total
CT = C // 128               # 16 contraction tiles
TCH = 512                   # phase-1 token chunk
NTCH = BT // TCH            # 8
Q = 512                     # phase-2 query chunk
NQC = T // Q                # 4 per (b, h)
EXPG = 2                    # k-tiles batched per exp instruction
P3CH = 256                  # phase-3 token chunk
OSL = C // NCORES           # 256 output channels per core


def build_nc(cc: bool = True):
    nc = bacc.Bacc("TRN2", target_bir_lowering=False, num_devices=NCORES)

    xT = nc.dram_tensor("xT", [C, BT], F32R, kind="ExternalInput")
    wqkvT = nc.dram_tensor("wqkvT", [C, 6 * HS], F32R, kind="ExternalInput")
    wprojT = nc.dram_tensor("wprojT", [C, OSL], F32R, kind="ExternalInput")
    masks = nc.dram_tensor("masks", [128, 4, Q], F32R, kind="ExternalInput")
    ones = nc.dram_tensor("ones", [128, 128], F32R, kind="ExternalInput")
    out_loc = nc.dram_tensor("out_loc", [BT, OSL], F32, kind="ExternalOutput")

    # per-batch spill tensors so batch-0 attention loads don't wait on
    # batch-1 projection writes
    q_dram = [nc.dram_tensor(f"q_dram{b}", [HPC * HS, T], F32R) for b in range(B)]
    k_dram = [nc.dram_tensor(f"k_dram{b}", [HPC * HS, T], F32R) for b in range(B)]
    v_dram = [nc.dram_tensor(f"v_dram{b}", [T, HPC * HS], F32R) for b in range(B)]
    y_loc = [nc.dram_tensor(f"y_loc{b}", [HPC * HS, T], F32R) for b in range(B)]
    yg = [
        nc.dram_tensor(f"yg{b}", [C, T], F32R, addr_space="Shared" if cc else "Local")
        for b in range(B)
    ]

    with tile.TileContext(nc) as tc:
        # ---------------- phase 1: QKV projection ----------------
        with (
            tc.tile_pool(name="wq", bufs=1) as wq_pool,
            tc.tile_pool(name="xin", bufs=3) as xin,
            tc.tile_pool(name="qkst", bufs=4) as qkst,
            tc.tile_pool(name="vst", bufs=4) as vst,
            tc.tile_pool(name="ps1", bufs=4, space="PSUM") as ps1,
            tc.tile_pool(name="psv", bufs=4, space="PSUM") as psv,
        ):
            wq_sb = wq_pool.tile([128, CT, 6 * HS], F32R)
            # split big loads into row-groups so the first matmuls start early
            for g in range(4):
                nc.sync.dma_start(
                    out=wq_sb[:, 4 * g : 4 * g + 4, :],
                    in_=wqkvT[512 * g : 512 * (g + 1), :].rearrange(
                        "(ko p) o -> p ko o", p=128
                    ),
                )
            for tch in range(NTCH):
                bb, tin = tch // (NTCH // B), (tch % (NTCH // B)) * TCH
                tsl = slice(tin, tin + TCH)
                x_sb = xin.tile([128, CT, TCH], F32R)
                for g in range(4):
                    nc.sync.dma_start(
                        out=x_sb[:, 4 * g : 4 * g + 4, :],
                        in_=xT[512 * g : 512 * (g + 1), bb * T + tin : bb * T + tin + TCH].rearrange(
                            "(ko p) t -> p ko t", p=128
                        ),
                    )
                for ot in range(4):  # q_h0, q_h1, k_h0, k_h1
                    pq = ps1.tile([128, TCH], F32)
                    for ci in range(CT):
                        nc.tensor.matmul(
                            pq[:],
                            wq_sb[:, ci, ot * 128 : (ot + 1) * 128],
                            x_sb[:, ci, :],
                            start=(ci == 0),
                            stop=(ci == CT - 1),
                        )
                    st = qkst.tile([128, TCH], F32R)
                    nc.vector.tensor_copy(out=st[:], in_=pq[:])
                    dst = (q_dram if ot < 2 else k_dram)[bb]
                    hl = ot % 2
                    nc.sync.dma_start(out=dst[hl * 128 : (hl + 1) * 128, tsl], in_=st[:])
                for tt in range(TCH // 128):  # V in natural [token, d] layout
                    pv = psv.tile([128, 2 * HS], F32)
                    for ci in range(CT):
                        nc.tensor.matmul(
                            pv[:],
                            x_sb[:, ci, tt * 128 : (tt + 1) * 128],
                            wq_sb[:, ci, 4 * HS : 6 * HS],
                            start=(ci == 0),
                            stop=(ci == CT - 1),
                        )
                    sv = vst.tile([128, 2 * HS], F32R)
                    nc.vector.tensor_copy(out=sv[:], in_=pv[:])
                    nc.sync.dma_start(
                        out=v_dram[bb][tin + tt * 128 : tin + (tt + 1) * 128, :],
                        in_=sv[:],
                    )

        # ---------------- phases 2+3: attention, gather, out-proj ----------------
        with (
            tc.tile_pool(name="const2", bufs=1) as const2,
            tc.tile_pool(name="wp", bufs=1) as wp_pool,
            tc.tile_pool(name="qp", bufs=2) as qp,
            tc.tile_pool(name="kp", bufs=2) as kp,
            tc.tile_pool(name="vp", bufs=2) as vp,
            tc.tile_pool(name="esp", bufs=2) as esp,
            tc.tile_pool(name="rp", bufs=2) as rp,
            tc.tile_pool(name="yst", bufs=2) as yst,
            tc.tile_pool(name="ygp", bufs=3) as ygp,
            tc.tile_pool(name="ost", bufs=2) as ost,
            tc.tile_pool(name="ps_s", bufs=2, space="PSUM") as ps_s,
            tc.tile_pool(name="ps_d", bufs=1, space="PSUM") as ps_d,
            tc.tile_pool(name="ps_o", bufs=2, space="PSUM") as ps_o,
            tc.tile_pool(name="ps3", bufs=1, space="PSUM") as ps3,
        ):
            # phase-2 loads go through GpSimd/SWDGE: the HWDGE queues are
            # backed up with phase-1 traffic, and these are dependency-ready
            # mid-phase-1
            masks_sb = const2.tile([128, 4, Q], F32R)
            nc.gpsimd.dma_start(out=masks_sb, in_=masks[:])
            ones_sb = const2.tile([128, 128], F32R)
            nc.gpsimd.dma_start(out=ones_sb, in_=ones[:])
            wp_sb = wp_pool.tile([128, CT, OSL], F32R)
            nc.gpsimd.dma_start(
                out=wp_sb, in_=wprojT.ap().rearrange("(ko p) o -> p ko o", p=128)
            )

            # denom/AV matmuls are emitted one chunk late so the in-order PE
            # queue has S-matmuls of the next chunk to chew on while the last
            # exp group of the current chunk drains through ACT/DVE
            pending: list = []

            def flush_pending():
                while pending:
                    pending.pop(0)()

            def attention_pair(b: int, hl: int):
                hsl = slice(hl * 128, (hl + 1) * 128)
                q_sb = qp.tile([128, T], F32R)
                nc.gpsimd.dma_start(out=q_sb, in_=q_dram[b][hsl, :])
                k_sb = kp.tile([128, CT, 128], F32R)
                nc.gpsimd.dma_start(
                    out=k_sb, in_=k_dram[b][hsl, :].rearrange("p (kt t) -> p kt t", t=128)
                )
                v_sb = vp.tile([128, CT, HS], F32R)
                nc.gpsimd.dma_start(
                    out=v_sb, in_=v_dram[b][:, hsl].rearrange("(kt p) d -> p kt d", p=128)
                )
                for qc in range(NQC):
                    nk = (qc + 1) * (Q // 128)  # causal: k-tiles 0..nk-1
                    qsl = slice(qc * Q, (qc + 1) * Q)
                    es = esp.tile([128, CT, Q], F32R)
                    for g in range(nk // EXPG):
                        sp = ps_s.tile([128, EXPG * Q], F32)
                        for j in range(EXPG):
                            kt = g * EXPG + j
                            nc.tensor.matmul(
                                sp[:, j * Q : (j + 1) * Q],
                                k_sb[:, kt, :],
                                q_sb[:, qsl],
                                start=True,
                                stop=True,
                            )
                        nc.scalar.activation(
                            out=es[:, g * EXPG : (g + 1) * EXPG, :].rearrange(
                                "p a q -> p (a q)"
                            ),
                            in_=sp[:],
                            func=EXP,
                        )
                        if g * EXPG >= nk - 4:  # diagonal groups -> 0/1 mask
                            a0 = g * EXPG - (nk - 4)
                            nc.vector.tensor_tensor(
                                es[:, g * EXPG : (g + 1) * EXPG, :],
                                es[:, g * EXPG : (g + 1) * EXPG, :],
                                masks_sb[:, a0 : a0 + EXPG, :],
                                mybir.AluOpType.mult,
                            )
                    flush_pending()
                    pending.append(
                        lambda b=b, hl=hl, qc=qc, nk=nk, es=es, v_sb=v_sb, hsl=hsl, qsl=qsl: denom_av(
                            b, hl, qc, nk, es, v_sb, hsl, qsl
                        )
                    )

            def denom_av(b, hl, qc, nk, es, v_sb, hsl, qsl):
                dp = ps_d.tile([128, Q], F32)
                for kt in range(nk):
                    nc.tensor.matmul(
                        dp[:], ones_sb[:], es[:, kt, :],
                        start=(kt == 0), stop=(kt == nk - 1),
                    )
                # 1/x as exp(-ln(x)) on ScalarE: DVE's reciprocal intrinsic
                # costs ~3.4us/tile and clogs the DVE queue
                ln_sb = rp.tile([128, Q], F32, tag="lnt", name="ln_sb")
                nc.scalar.activation(out=ln_sb[:], in_=dp[:], func=LN)
                r_sb = rp.tile([128, Q], F32, tag="rsb", name="r_sb")
                nc.scalar.activation(out=r_sb[:], in_=ln_sb[:], func=EXP, scale=-1.0)
                po = ps_o.tile([128, Q], F32)
                for kt in range(nk):
                    nc.tensor.matmul(
                        po[:], v_sb[:, kt, :], es[:, kt, :],
                        start=(kt == 0), stop=(kt == nk - 1),
                    )
                y_sb = yst.tile([128, Q], F32R)
                nc.vector.tensor_mul(out=y_sb[:], in0=po[:], in1=r_sb[:])
                nc.sync.dma_start(out=y_loc[b][hsl, qsl], in_=y_sb[:])

            def gather(b: int):
                if cc:
                    nc.gpsimd.collective_compute(
                        "AllGather",
                        mybir.AluOpType.bypass,
                        replica_groups=[list(range(NCORES))],
                        ins=[y_loc[b].ap()],
                        outs=[yg[b].ap()],
                    )
                else:  # timing-only variant: no inter-core traffic
                    nc.sync.dma_start(out=yg[b][: HPC * HS, :], in_=y_loc[b].ap())

            def out_proj(b: int):
                for ch in range(T // P3CH):
                    csl = slice(ch * P3CH, (ch + 1) * P3CH)
                    yg_sb = ygp.tile([128, CT, P3CH], F32R)
                    nc.sync.dma_start(
                        out=yg_sb, in_=yg[b][:, csl].rearrange("(ko p) t -> p ko t", p=128)
                    )
                    for tt in range(P3CH // 128):
                        po = ps3.tile([128, OSL], F32)
                        for ci in range(CT):
                            nc.tensor.matmul(
                                po[:],
                                yg_sb[:, ci, tt * 128 : (tt + 1) * 128],
                                wp_sb[:, ci, :],
                                start=(ci == 0),
                                stop=(ci == CT - 1),
                            )
                        o_sb = ost.tile([128, OSL], F32)
                        nc.vector.tensor_copy(out=o_sb[:], in_=po[:])
                        nc.sync.dma_start(
                            out=out_loc[b * T + ch * P3CH + tt * 128 : b * T + ch * P3CH + (tt + 1) * 128, :],
                            in_=o_sb[:],
                        )

            attention_pair(0, 0)
            attention_pair(0, 1)
            flush_pending()
            gather(0)
            attention_pair(1, 0)
            attention_pair(1, 1)
            flush_pending()
            gather(1)
            out_proj(0)
            out_proj(1)

    nc.finalize()
    return nc


def prep_inputs(x: np.ndarray, w_attn: np.ndarray, w_proj: np.ndarray):
    """Host-side sharding/layout. Returns per-core input maps."""
    xT = np.ascontiguousarray(x.reshape(BT, C).T)
    wq, wk, wv = w_attn[:C], w_attn[C : 2 * C], w_attn[2 * C :]
    scale = np.float32(1.0 / np.sqrt(HS))
    kk = np.arange(128, dtype=np.int64)[:, None, None]
    aa = np.arange(4, dtype=np.int64)[None, :, None]
    qq = np.arange(Q, dtype=np.int64)[None, None, :]
    masks = (128 * aa + kk <= qq).astype(np.float32)
    in_maps = []
    for c in range(NCORES):
        h0 = HPC * c
        rows = slice(h0 * HS, (h0 + HPC) * HS)
        wqkvT = np.ascontiguousarray(
            np.concatenate([wq[rows] * scale, wk[rows], wv[rows]], axis=0).T
        )
        wprojT = np.ascontiguousarray(w_proj[c * OSL : (c + 1) * OSL, :].T)
        in_maps.append(
            {
                "xT": xT,
                "wqkvT": wqkvT,
                "wprojT": wprojT,
                "masks": masks,
                "ones": np.ones((128, 128), dtype=np.float32),
            }
        )
    return in_maps


_CACHE: dict = {}


def _get_nc(cc: bool = True):
    key = ("nc", cc)
    if key not in _CACHE:
        _CACHE[key] = build_nc(cc=cc)
    return _CACHE[key]


def run(x, w_attn, w_proj, cc: bool = True, **spmd_kwargs):
    nc = _get_nc(cc=cc)
    in_maps = prep_inputs(
        np.asarray(x, dtype=np.float32),
        np.asarray(w_attn, dtype=np.float32),
        np.asarray(w_proj, dtype=np.float32),
    )
    res = run_bass_kernel_spmd(nc, in_maps, list(range(NCORES)), **spmd_kwargs)
    out = np.concatenate([res.results[c]["out_loc"] for c in range(NCORES)], axis=1)
    return out.reshape(B, T, C), res


def kernel(x, w_attn, w_proj):
    out, _ = run(x, w_attn, w_proj, cc=True)
    return out
